# revision 15
# baseline (speedup 1.0000x reference)
"""FourierBlock kernel for 8 TRN2 NeuronCores.

Math: the reference keeps only the first 64 rfft modes, so the whole op is
    out[b] = CS @ Y2[b],  Y2 = mode-mix(X2, W),  X2 = F2 @ q[b]
with F2 [128,4096] = [cos; -sin] forward-DFT rows and CS the inverse-DFT
columns (factor 2/L, except DC).  The complex combine (Yr = XrWr - XiWi etc.)
is folded into step-3's coefficient matrix CS4 [4096, 256] acting on the four
uncombined product groups P = [XrWr; XiWr; XrWi; XiWi].

Sharding: core c owns batch c for steps 1/3 (data parallel) and modes
[8c, 8c+8) for step 2 (tensor parallel over modes -> W is read exactly once
across the chip).  Two AllToAlls exchange the small X2 / P intermediates.
"""

import numpy as np

B, L, D, M = 8, 4096, 512, 64
NCORES = 8
T = M // NCORES  # local modes per core


def _constants():
    k = np.arange(M)
    l = np.arange(L)
    ang = 2 * np.pi * np.outer(k, l) / L  # [M, L]
    # Row order (s, a, t): partition p = s*16 + a*8 + t holds
    # cos (a=0) / -sin (a=1) of mode k = 8s + t, so x2's partition layout
    # already equals the AllToAll bounce layout [s][a, t] (straight DMA).
    F2 = np.stack([np.cos(ang), -np.sin(ang)], axis=0)  # [2, M, L]
    F2 = F2.reshape(2, NCORES, T, L).transpose(1, 0, 2, 3).reshape(128, L)
    fmat = np.ascontiguousarray(F2.T, dtype=np.float32)  # [L, 128] (lhsT layout)

    ck = np.where(k == 0, 1.0, 2.0) / L
    ang2 = 2 * np.pi * np.outer(l, k) / L  # [L, M]
    C = ck * np.cos(ang2)  # [L, M]
    S = -(2.0 / L) * np.sin(ang2)
    # CS4 [L, (j, g, t)] with g: 0=XrWr->C, 1=XiWr->S, 2=XrWi->S, 3=XiWi->-C
    Cr = C.reshape(L, NCORES, T)
    Sr = S.reshape(L, NCORES, T)
    CS4 = np.stack([Cr, Sr, Sr, -Cr], axis=2)  # [L, NCORES, 4, T]
    # lhsT layout: [2 kc, 128, L]  (K=256 on partitions, 2 chunks)
    cmat = np.ascontiguousarray(
        CS4.reshape(L, 256).T.reshape(2, 128, L), dtype=np.float32
    )
    return fmat, cmat


def build_nc(debug=False):
    import concourse.bacc as bacc
    import concourse.mybir as mybir
    import concourse.tile as tile

    f32 = mybir.dt.float32
    f32r = mybir.dt.float32r
    nc = bacc.Bacc("TRN2", target_bir_lowering=False, num_devices=NCORES)

    qb = nc.dram_tensor("qb", [L, D], f32r, kind="ExternalInput")
    wr = nc.dram_tensor("wr", [T, D, D], f32r, kind="ExternalInput")
    wi = nc.dram_tensor("wi", [T, D, D], f32r, kind="ExternalInput")
    out = nc.dram_tensor("out", [L, D], f32, kind="ExternalOutput")

    fmat_d = nc.dram_tensor("fmat", [L, 128], f32r, kind="ExternalInput")
    cmat_d = nc.dram_tensor("cmat", [2, 128, L], f32r, kind="ExternalInput")
    ident_d = nc.dram_tensor("ident", [128, 128], f32r, kind="ExternalInput")
    if debug:
        dbg_x2 = nc.dram_tensor("dbg_x2", [128, 512], f32r, kind="ExternalOutput")
        dbg_xm = nc.dram_tensor("dbg_xm", [128, 512], f32r, kind="ExternalOutput")
        dbg_xt = nc.dram_tensor("dbg_xt", [128, 512], f32r, kind="ExternalOutput")
        dbg_stage = nc.dram_tensor("dbg_stage", [16, 8192], f32r, kind="ExternalOutput")
        dbg_p = nc.dram_tensor("dbg_p", [2, 128, 512], f32r, kind="ExternalOutput")

    RG = [list(range(NCORES))]

    with tile.TileContext(nc) as tc:
        with (
            tc.tile_pool(name="constp", bufs=1) as constp,
            tc.tile_pool(name="qpool", bufs=4) as qpool,
            tc.tile_pool(name="wpool", bufs=2) as wpool,
            tc.tile_pool(name="misc", bufs=1) as misc,
            tc.tile_pool(name="outp", bufs=4) as outp,
            tc.tile_pool(name="pacc", bufs=2, space="PSUM") as pacc,
            tc.tile_pool(name="ptp", bufs=2, space="PSUM") as ptp,
            tc.tile_pool(name="po", bufs=4, space="PSUM") as po,
            tc.tile_pool(name="dram", bufs=1, space="DRAM") as dram,
        ):
            # constants into SBUF
            fmat_sb = constp.tile([128, 32 * 128], f32r)
            nc.sync.dma_start(
                out=fmat_sb[:].rearrange("p (n m) -> p n m", n=32),
                in_=fmat_d[:].rearrange("(n p) m -> p n m", p=128),
            )
            cmat_sb = constp.tile([128, 2 * L], f32r)
            nc.sync.dma_start(
                out=cmat_sb[:].rearrange("p (k m) -> p k m", k=2),
                in_=cmat_d[:].rearrange("k p m -> p k m"),
            )
            ident_sb = constp.tile([128, 128], f32r)
            nc.sync.dma_start(out=ident_sb[:], in_=ident_d[:])

            # ---- step 1: X2 = F2 @ qb  -> [128 (a,s,t), 512 d]
            x2ps = pacc.tile([128, 512], f32, tag="acc")
            for li in range(32):
                qt = qpool.tile([128, 512], f32r, name="qt", tag="qt")
                nc.sync.dma_start(out=qt[:], in_=qb[li * 128 : (li + 1) * 128, :])
                nc.tensor.matmul(
                    x2ps[:],
                    lhsT=fmat_sb[:, li * 128 : (li + 1) * 128],
                    rhs=qt[:],
                    start=(li == 0),
                    stop=(li == 31),
                )
            x2sb = misc.tile([128, 512], f32r)
            nc.scalar.copy(x2sb[:], x2ps[:])
            if debug:
                nc.sync.dma_start(out=dbg_x2[:], in_=x2sb[:])

            # ---- exchange 1: shard s gets my batch's X rows for modes of core s
            # (x2 partition order is (s, a, t) by construction of fmat)
            b1in = dram.tile([NCORES, 2, T, D], f32r)
            nc.sync.dma_start(
                out=b1in[:].rearrange("s a t d -> (s a t) d"), in_=x2sb[:]
            )
            b1out = dram.tile([NCORES, 2, T, D], f32r)
            nc.gpsimd.collective_compute(
                "AllToAll",
                mybir.AluOpType.bypass,
                replica_groups=RG,
                ins=[b1in.opt()],
                outs=[b1out.opt()],
            )

            # ---- load Xm [128 (j,a,t), 512 d] and transpose to [128 d, (dc,j,a,t)]
            xm_sb = misc.tile([128, 512], f32r)
            nc.sync.dma_start(
                out=xm_sb[:], in_=b1out[:].rearrange("j a t d -> (j a t) d")
            )
            if debug:
                nc.sync.dma_start(out=dbg_xm[:], in_=xm_sb[:])
            xt_sb = misc.tile([128, 512], f32r)
            for dc in range(4):
                tp = ptp.tile([128, 128], f32r, name="tp", tag="tp")
                nc.tensor.transpose(
                    tp[:], xm_sb[:, dc * 128 : (dc + 1) * 128], ident_sb[:]
                )
                cp = nc.scalar.copy if dc % 2 else nc.vector.tensor_copy
                cp(xt_sb[:, dc * 128 : (dc + 1) * 128], tp[:])

            # ---- step 2: per local mode t, products vs Wr/Wi.
            # M=16 columns m = 2j+a (single stride-8 free dim, as the weight AP
            # must be 1-D): psum row m -> (batch j, re/im a) interleaved.
            stage = misc.tile([16, 2 * T * D], f32r)  # [16, 8192] = (g2, t, e)
            xt_v = xt_sb[:].rearrange("p (dc m t) -> p dc m t", dc=4, m=16, t=T)
            for t in range(T):
                wrt = wpool.tile([128, 4 * 512], f32r, name="wrt", tag="wrt")
                nc.sync.dma_start(
                    out=wrt[:].rearrange("p (n e) -> p n e", n=4),
                    in_=wr[t].rearrange("(n p) e -> p n e", p=128),
                )
                wit = wpool.tile([128, 4 * 512], f32r, name="wit", tag="wit")
                nc.sync.dma_start(
                    out=wit[:].rearrange("p (n e) -> p n e", n=4),
                    in_=wi[t].rearrange("(n p) e -> p n e", p=128),
                )
                o1 = po.tile([16, 512], f32, name="o1", tag="o")
                o2 = po.tile([16, 512], f32, name="o2", tag="o")
                for dc in range(4):
                    nc.tensor.matmul(
                        o1[:],
                        lhsT=xt_v[:, dc, :, t],
                        rhs=wrt[:, dc * 512 : (dc + 1) * 512],
                        start=(dc == 0),
                        stop=(dc == 3),
                    )
                for dc in range(4):
                    nc.tensor.matmul(
                        o2[:],
                        lhsT=xt_v[:, dc, :, t],
                        rhs=wit[:, dc * 512 : (dc + 1) * 512],
                        start=(dc == 0),
                        stop=(dc == 3),
                    )
                cp1 = nc.scalar.copy if t % 2 else nc.vector.tensor_copy
                cp2 = nc.vector.tensor_copy if t % 2 else nc.scalar.copy
                cp1(stage[:, t * 512 : (t + 1) * 512], o1[:])
                cp2(stage[:, T * D + t * 512 : T * D + (t + 1) * 512], o2[:])

            # ---- exchange 2: shard j gets products for batch j (my modes)
            if debug:
                nc.sync.dma_start(out=dbg_xt[:], in_=xt_sb[:])
                nc.sync.dma_start(out=dbg_stage[:], in_=stage[:])
            # b2in layout [j, a, g2, t, e]: shard j = stage partitions 2j..2j+1,
            # each partition's free dim (g2, t, e) lands contiguously.
            b2in = dram.tile([NCORES, 2, 2, T, D], f32r)
            nc.sync.dma_start(
                out=b2in[:].rearrange("j a g2 t e -> (j a) (g2 t e)"),
                in_=stage[:],
            )
            b2out = dram.tile([NCORES, 2, 2, T, D], f32r)
            nc.gpsimd.collective_compute(
                "AllToAll",
                mybir.AluOpType.bypass,
                replica_groups=RG,
                ins=[b2in.opt()],
                outs=[b2out.opt()],
            )

            # ---- step 3: out = CS4 @ P, K=256 in 2 chunks of 128 (rows (j,g,t))
            p0 = misc.tile([128, 512], f32r, name="p0")
            nc.sync.dma_start(
                out=p0[:], in_=b2out[0:4].rearrange("j a g t e -> (j a g t) e")
            )
            p1 = misc.tile([128, 512], f32r, name="p1")
            nc.sync.dma_start(
                out=p1[:], in_=b2out[4:8].rearrange("j a g t e -> (j a g t) e")
            )
            if debug:
                nc.sync.dma_start(out=dbg_p[0], in_=p0[:])
                nc.sync.dma_start(out=dbg_p[1], in_=p1[:])
            for m in range(32):
                ps = pacc.tile([128, 512], f32, name="ps3", tag="acc")
                nc.tensor.matmul(
                    ps[:],
                    lhsT=cmat_sb[:, m * 128 : (m + 1) * 128],
                    rhs=p0[:],
                    start=True,
                    stop=False,
                )
                nc.tensor.matmul(
                    ps[:],
                    lhsT=cmat_sb[:, L + m * 128 : L + (m + 1) * 128],
                    rhs=p1[:],
                    start=False,
                    stop=True,
                )
                ot = outp.tile([128, 512], f32, name="ot", tag="ot")
                cp = nc.scalar.copy if m % 2 else nc.vector.tensor_copy
                cp(ot[:], ps[:])
                nc.sync.dma_start(out=out[m * 128 : (m + 1) * 128, :], in_=ot[:])

    nc.compile()
    return nc


_NC_CACHE = None


def _get_nc():
    global _NC_CACHE
    if _NC_CACHE is None:
        _NC_CACHE = build_nc()
    return _NC_CACHE


def run(q, w_real, w_imag, trace=False, debug=False):
    from concourse.bass_utils import run_bass_kernel_spmd

    nc = build_nc(debug=True) if debug else _get_nc()
    q = np.ascontiguousarray(np.asarray(q), dtype=np.float32)
    w_real = np.asarray(w_real)
    w_imag = np.asarray(w_imag)
    fmat_np, cmat_np = _constants()
    ident_np = np.eye(128, dtype=np.float32)
    in_maps = []
    for c in range(NCORES):
        sl = slice(c * T, (c + 1) * T)
        in_maps.append(
            {
                "qb": np.ascontiguousarray(q[c]),
                "wr": np.ascontiguousarray(
                    np.transpose(w_real[:, :, sl], (2, 0, 1)), dtype=np.float32
                ),
                "wi": np.ascontiguousarray(
                    np.transpose(w_imag[:, :, sl], (2, 0, 1)), dtype=np.float32
                ),
                "fmat": fmat_np,
                "cmat": cmat_np,
                "ident": ident_np,
            }
        )
    res = run_bass_kernel_spmd(
        nc, in_maps, core_ids=list(range(NCORES)), trace=trace
    )
    out = np.stack([r["out"] for r in res.results], axis=0)
    return out, res


def kernel(q, w_real, w_imag):
    out, _ = run(q, w_real, w_imag)
    return out


# revision 16
# speedup vs baseline: 1.0308x; 1.0308x over previous
"""FourierBlock kernel for 8 TRN2 NeuronCores.

Math: the reference keeps only the first 64 rfft modes, so the whole op is
    out[b] = CS @ Y2[b],  Y2 = mode-mix(X2, W),  X2 = F2 @ q[b]
with F2 [128,4096] = [cos; -sin] forward-DFT rows and CS the inverse-DFT
columns (factor 2/L, except DC).  The complex combine (Yr = XrWr - XiWi etc.)
is folded into step-3's coefficient matrix CS4 [4096, 256] acting on the four
uncombined product groups (XrWr, XiWr, XrWi, XiWi).

Sharding: core c owns batch c for steps 1/3 (data parallel) and modes
[8c, 8c+8) for step 2 (tensor parallel over modes -> W is read exactly once
across the chip).  AllToAlls exchange the small X2 / product intermediates;
the second exchange is split by product group (Wr then Wi) so the first
half overlaps the second half of step 2.

All matmul operands are float32r (FP22 single-pass PE mode, ~2.5e-4 rel err).
Host pre-arranges W/fmat so every big DMA is fully contiguous.
"""

import numpy as np

B, L, D, M = 8, 4096, 512, 64
NCORES = 8
T = M // NCORES  # local modes per core


def _constants():
    k = np.arange(M)
    l = np.arange(L)
    ang = 2 * np.pi * np.outer(k, l) / L  # [M, L]
    # F2 row order (s, a, t): partition p = s*16 + a*8 + t holds
    # cos (a=0) / -sin (a=1) of mode k = 8s + t, so x2's partition layout
    # already equals the AllToAll bounce layout [s][a, t] (straight DMA).
    F2 = np.stack([np.cos(ang), -np.sin(ang)], axis=0)  # [2, M, L]
    F2 = F2.reshape(2, NCORES, T, L).transpose(1, 0, 2, 3).reshape(128, L)
    # lhsT chunks, p-major for contiguous DMA: fmat[p, n, m] = F2[m, n*128+p]
    fmat = np.ascontiguousarray(
        F2.T.reshape(32, 128, 128).transpose(1, 0, 2), dtype=np.float32
    )  # [128, 32, 128]

    ck = np.where(k == 0, 1.0, 2.0) / L
    ang2 = 2 * np.pi * np.outer(l, k) / L  # [L, M]
    C = (ck * np.cos(ang2)).reshape(L, NCORES, T)
    S = (-(2.0 / L) * np.sin(ang2)).reshape(L, NCORES, T)
    # K order (g2, j, a, t): chunk g2=0 (Wr products): a=0 -> C, a=1 -> S;
    # chunk g2=1 (Wi products): a=0 -> S, a=1 -> -C.
    ch0 = np.stack([C, S], axis=2).reshape(L, 128)  # [L, (j, a, t)]
    ch1 = np.stack([S, -C], axis=2).reshape(L, 128)
    cmat = np.ascontiguousarray(
        np.stack([ch0.T, ch1.T], axis=0), dtype=np.float32
    )  # [2, 128, L]
    return fmat, cmat


def build_nc(debug=False):
    import concourse.bacc as bacc
    import concourse.mybir as mybir
    import concourse.tile as tile

    f32 = mybir.dt.float32
    f32r = mybir.dt.float32r
    nc = bacc.Bacc("TRN2", target_bir_lowering=False, num_devices=NCORES)

    qb = nc.dram_tensor("qb", [L, D], f32r, kind="ExternalInput")
    # W pre-arranged on host: w[g2][t, p, dc, e] = W_g2[dc*128+p, e, 8c+t]
    wr = nc.dram_tensor("wr", [T, 128, 4, 512], f32r, kind="ExternalInput")
    wi = nc.dram_tensor("wi", [T, 128, 4, 512], f32r, kind="ExternalInput")
    out = nc.dram_tensor("out", [L, D], f32, kind="ExternalOutput")

    fmat_d = nc.dram_tensor("fmat", [128, 32, 128], f32r, kind="ExternalInput")
    cmat_d = nc.dram_tensor("cmat", [2, 128, L], f32r, kind="ExternalInput")
    ident_d = nc.dram_tensor("ident", [128, 128], f32r, kind="ExternalInput")
    if debug:
        dbg_x2 = nc.dram_tensor("dbg_x2", [128, 512], f32r, kind="ExternalOutput")
        dbg_xm = nc.dram_tensor("dbg_xm", [128, 512], f32r, kind="ExternalOutput")
        dbg_xt = nc.dram_tensor("dbg_xt", [128, 512], f32r, kind="ExternalOutput")
        dbg_stage = nc.dram_tensor(
            "dbg_stage", [2, 16, 4096], f32r, kind="ExternalOutput"
        )
        dbg_p = nc.dram_tensor("dbg_p", [2, 128, 512], f32r, kind="ExternalOutput")

    RG = [list(range(NCORES))]

    with tile.TileContext(nc) as tc:
        with (
            tc.tile_pool(name="constp", bufs=1) as constp,
            tc.tile_pool(name="qpool", bufs=3) as qpool,
            tc.tile_pool(name="wpool", bufs=4) as wpool,
            tc.tile_pool(name="misc", bufs=1) as misc,
            tc.tile_pool(name="outp", bufs=3) as outp,
            tc.tile_pool(name="pacc", bufs=2, space="PSUM") as pacc,
            tc.tile_pool(name="ptp", bufs=2, space="PSUM") as ptp,
            tc.tile_pool(name="po", bufs=4, space="PSUM") as po,
            tc.tile_pool(name="dram", bufs=1, space="DRAM") as dram,
        ):
            # fmat + ident on the sync ring (needed immediately)
            fmat_sb = constp.tile([128, 32 * 128], f32r)
            nc.sync.dma_start(
                out=fmat_sb[:].rearrange("p (n m) -> p n m", n=32), in_=fmat_d[:]
            )
            ident_sb = constp.tile([128, 128], f32r)
            nc.sync.dma_start(out=ident_sb[:], in_=ident_d[:])

            # W stream on the ACT ring: fully contiguous 1 MB transfers; runs
            # from t=0 and keeps streaming through the A2A1 stall.
            w_tiles = []
            for g2 in range(2):
                wsrc = wr if g2 == 0 else wi
                for t in range(T):
                    wt = wpool.tile(
                        [128, 4 * 512], f32r, name=f"w{g2}_{t}", tag=f"w{g2}"
                    )
                    nc.scalar.dma_start(
                        out=wt[:].rearrange("p (n e) -> p n e", n=4), in_=wsrc[t]
                    )
                    w_tiles.append(wt)

            # cmat on the ACT ring after W (needed only for step 3)
            cmat_sb = constp.tile([128, 2 * L], f32r)
            nc.scalar.dma_start(
                out=cmat_sb[:].rearrange("p (k m) -> p k m", k=2),
                in_=cmat_d[:].rearrange("k p m -> p k m"),
            )

            # ---- step 1: X2 = F2 @ qb  -> [128 (s,a,t), 512 d]
            x2ps = pacc.tile([128, 512], f32, tag="acc")
            for lo in range(8):  # 1 MB q transfers, 4 l-chunks each
                qt = qpool.tile([128, 4 * 512], f32r, name="qt", tag="qt")
                nc.sync.dma_start(
                    out=qt[:].rearrange("p (n d) -> p n d", n=4),
                    in_=qb[:].rearrange("(n p) d -> p n d", p=128)[
                        :, lo * 4 : (lo + 1) * 4
                    ],
                )
                for li in range(4):
                    gl = lo * 4 + li
                    nc.tensor.matmul(
                        x2ps[:],
                        lhsT=fmat_sb[:, gl * 128 : (gl + 1) * 128],
                        rhs=qt[:, li * 512 : (li + 1) * 512],
                        start=(gl == 0),
                        stop=(gl == 31),
                    )
            x2sb = misc.tile([128, 512], f32r)
            nc.scalar.copy(x2sb[:], x2ps[:])
            if debug:
                nc.sync.dma_start(out=dbg_x2[:], in_=x2sb[:])

            # ---- exchange 1: shard s = my batch's X rows for core s's modes
            b1in = dram.tile([NCORES, 2, T, D], f32r)
            nc.sync.dma_start(
                out=b1in[:].rearrange("s a t d -> (s a t) d"), in_=x2sb[:]
            )
            b1out = dram.tile([NCORES, 2, T, D], f32r)
            nc.gpsimd.collective_compute(
                "AllToAll",
                mybir.AluOpType.bypass,
                replica_groups=RG,
                ins=[b1in.opt()],
                outs=[b1out.opt()],
            )

            # ---- load Xm [128 (j,a,t), 512 d], transpose to [128 d%, (dc,j,a,t)]
            xm_sb = misc.tile([128, 512], f32r)
            nc.sync.dma_start(
                out=xm_sb[:], in_=b1out[:].rearrange("j a t d -> (j a t) d")
            )
            if debug:
                nc.sync.dma_start(out=dbg_xm[:], in_=xm_sb[:])
            xt_sb = misc.tile([128, 512], f32r)
            for dc in range(4):
                tp = ptp.tile([128, 128], f32r, name="tp", tag="tp")
                nc.tensor.transpose(
                    tp[:], xm_sb[:, dc * 128 : (dc + 1) * 128], ident_sb[:]
                )
                cp = nc.scalar.copy if dc % 2 else nc.vector.tensor_copy
                cp(xt_sb[:, dc * 128 : (dc + 1) * 128], tp[:])

            # ---- step 2: all Wr products first, then all Wi products.
            # M=16 columns m = 2j+a (single stride-8 free dim).
            xt_v = xt_sb[:].rearrange("p (dc m t) -> p dc m t", dc=4, m=16, t=T)
            stages = []
            b2outs = []
            for g2 in range(2):
                stage = misc.tile([16, T * D], f32r, name=f"stage{g2}")
                stages.append(stage)
                for t in range(T):
                    wt = w_tiles[g2 * T + t]
                    o = po.tile([16, 512], f32, name="o", tag="o")
                    for dc in range(4):
                        nc.tensor.matmul(
                            o[:],
                            lhsT=xt_v[:, dc, :, t],
                            rhs=wt[:, dc * 512 : (dc + 1) * 512],
                            start=(dc == 0),
                            stop=(dc == 3),
                        )
                    cp = nc.scalar.copy if t % 2 else nc.vector.tensor_copy
                    cp(stage[:, t * 512 : (t + 1) * 512], o[:])
                if debug:
                    nc.sync.dma_start(out=dbg_stage[g2], in_=stage[:])

                # exchange 2 (per product group): shard j = partitions 2j..2j+1
                b2in = dram.tile([NCORES, 2, T, D], f32r, name=f"b2in{g2}")
                nc.sync.dma_start(
                    out=b2in[:].rearrange("j a t e -> (j a) (t e)"), in_=stage[:]
                )
                b2out = dram.tile([NCORES, 2, T, D], f32r, name=f"b2out{g2}")
                nc.gpsimd.collective_compute(
                    "AllToAll",
                    mybir.AluOpType.bypass,
                    replica_groups=RG,
                    ins=[b2in.opt()],
                    outs=[b2out.opt()],
                )
                b2outs.append(b2out)

            # ---- step 3: out = CS4 @ P, K = 2 chunks of 128 (g2-major)
            ps_rhs = []
            for g2 in range(2):
                pg = misc.tile([128, 512], f32r, name=f"pg{g2}")
                nc.sync.dma_start(
                    out=pg[:],
                    in_=b2outs[g2][:].rearrange("j a t e -> (j a t) e"),
                )
                ps_rhs.append(pg)
            if debug:
                nc.sync.dma_start(out=dbg_p[0], in_=ps_rhs[0][:])
                nc.sync.dma_start(out=dbg_p[1], in_=ps_rhs[1][:])
            for m in range(32):
                ps = pacc.tile([128, 512], f32, name="ps3", tag="acc")
                for kc in range(2):
                    nc.tensor.matmul(
                        ps[:],
                        lhsT=cmat_sb[:, kc * L + m * 128 : kc * L + (m + 1) * 128],
                        rhs=ps_rhs[kc][:],
                        start=(kc == 0),
                        stop=(kc == 1),
                    )
                ot = outp.tile([128, 512], f32, name="ot", tag="ot")
                cp = nc.scalar.copy if m % 2 else nc.vector.tensor_copy
                cp(ot[:], ps[:])
                nc.sync.dma_start(out=out[m * 128 : (m + 1) * 128, :], in_=ot[:])

    nc.compile()
    return nc


_NC_CACHE = None


def _get_nc():
    global _NC_CACHE
    if _NC_CACHE is None:
        _NC_CACHE = build_nc()
    return _NC_CACHE


def _prep_w(w, sl):
    # [D, D, M] -> slice modes -> [T, 128, 4, 512] with w[t, p, dc, e]
    wt = np.transpose(w[:, :, sl], (2, 0, 1))  # [T, d, e]
    wt = wt.reshape(T, 4, 128, 512).transpose(0, 2, 1, 3)  # [T, p, dc, e]
    return np.ascontiguousarray(wt, dtype=np.float32)


def run(q, w_real, w_imag, trace=False, debug=False):
    from concourse.bass_utils import run_bass_kernel_spmd

    nc = build_nc(debug=True) if debug else _get_nc()
    q = np.ascontiguousarray(np.asarray(q), dtype=np.float32)
    w_real = np.asarray(w_real)
    w_imag = np.asarray(w_imag)
    fmat_np, cmat_np = _constants()
    ident_np = np.eye(128, dtype=np.float32)
    in_maps = []
    for c in range(NCORES):
        sl = slice(c * T, (c + 1) * T)
        in_maps.append(
            {
                "qb": np.ascontiguousarray(q[c]),
                "wr": _prep_w(w_real, sl),
                "wi": _prep_w(w_imag, sl),
                "fmat": fmat_np,
                "cmat": cmat_np,
                "ident": ident_np,
            }
        )
    res = run_bass_kernel_spmd(
        nc, in_maps, core_ids=list(range(NCORES)), trace=trace
    )
    out = np.stack([r["out"] for r in res.results], axis=0)
    return out, res


def kernel(q, w_real, w_imag):
    out, _ = run(q, w_real, w_imag)
    return out


# revision 17
# speedup vs baseline: 1.0745x; 1.0423x over previous
"""FourierBlock kernel for 8 TRN2 NeuronCores.

Math: the reference keeps only the first 64 rfft modes, so the whole op is
    out[b] = CS @ Y2[b],  Y2 = mode-mix(X2, W),  X2 = F2 @ q[b]
with F2 [128,4096] = [cos; -sin] forward-DFT rows and CS the inverse-DFT
columns (factor 2/L, except DC).  The complex combine (Yr = XrWr - XiWi etc.)
is folded into step-3's coefficient matrix CS4 [4096, 256] acting on the four
uncombined product groups (XrWr, XiWr, XrWi, XiWi).

Sharding: core c owns batch c for steps 1/3 (data parallel) and modes
[8c, 8c+8) for step 2 (tensor parallel over modes -> W is read exactly once
across the chip).  Two AllToAlls exchange the small X2 / product tensors.

Precision: step 1 runs in float32r (FP22), steps 2/3 in bf16 with fp32
accumulation (~3e-3 rel err total).  W is cast f32->bf16 during the DMA
(SWDGE) so the whole 8 MB bf16 W slice stays resident in SBUF -- the W
stream never stalls on pool slots and fully overlaps the first AllToAll.
"""

import numpy as np

B, L, D, M = 8, 4096, 512, 64
NCORES = 8
T = M // NCORES  # local modes per core


def _constants():
    import ml_dtypes

    k = np.arange(M)
    l = np.arange(L)
    ang = 2 * np.pi * np.outer(k, l) / L  # [M, L]
    # F2 row order (s, a, t): partition p = s*16 + a*8 + t holds
    # cos (a=0) / -sin (a=1) of mode k = 8s + t, so x2's partition layout
    # already equals the AllToAll bounce layout [s][a, t] (straight DMA).
    F2 = np.stack([np.cos(ang), -np.sin(ang)], axis=0)  # [2, M, L]
    F2 = F2.reshape(2, NCORES, T, L).transpose(1, 0, 2, 3).reshape(128, L)
    # lhsT chunks, p-major for contiguous DMA: fmat[p, n, m] = F2[m, n*128+p]
    fmat = np.ascontiguousarray(
        F2.T.reshape(32, 128, 128).transpose(1, 0, 2), dtype=np.float32
    )  # [128, 32, 128]

    ck = np.where(k == 0, 1.0, 2.0) / L
    ang2 = 2 * np.pi * np.outer(l, k) / L  # [L, M]
    C = (ck * np.cos(ang2)).reshape(L, NCORES, T)
    S = (-(2.0 / L) * np.sin(ang2)).reshape(L, NCORES, T)
    # K order (j, a, g2, t): (a0,g0)=rWr->C, (a0,g1)=rWi->S,
    # (a1,g0)=iWr->S, (a1,g1)=iWi->-C
    CS4 = np.empty((L, NCORES, 2, 2, T))
    CS4[:, :, 0, 0] = C
    CS4[:, :, 0, 1] = S
    CS4[:, :, 1, 0] = S
    CS4[:, :, 1, 1] = -C
    cmat = np.ascontiguousarray(
        CS4.reshape(L, 256).T.reshape(2, 128, L).astype(ml_dtypes.bfloat16)
    )  # [2, 128, L] bf16
    return fmat, cmat


def build_nc(debug=False):
    import concourse.bacc as bacc
    import concourse.mybir as mybir
    import concourse.tile as tile

    f32 = mybir.dt.float32
    f32r = mybir.dt.float32r
    bf16 = mybir.dt.bfloat16
    nc = bacc.Bacc("TRN2", target_bir_lowering=False, num_devices=NCORES)

    qb = nc.dram_tensor("qb", [L, D], f32r, kind="ExternalInput")
    # W pre-arranged on host: w[g2][p, t, dc, e] = W_g2[dc*128+p, e, 8c+t]
    wr = nc.dram_tensor("wr", [128, T, 4, 512], f32, kind="ExternalInput")
    wi = nc.dram_tensor("wi", [128, T, 4, 512], f32, kind="ExternalInput")
    out = nc.dram_tensor("out", [L, D], f32, kind="ExternalOutput")

    fmat_d = nc.dram_tensor("fmat", [128, 32, 128], f32r, kind="ExternalInput")
    cmat_d = nc.dram_tensor("cmat", [2, 128, L], bf16, kind="ExternalInput")
    ident_d = nc.dram_tensor("ident", [128, 128], f32r, kind="ExternalInput")
    if debug:
        dbg_x2 = nc.dram_tensor("dbg_x2", [128, 512], f32r, kind="ExternalOutput")
        dbg_xm = nc.dram_tensor("dbg_xm", [128, 512], f32r, kind="ExternalOutput")
        dbg_stage = nc.dram_tensor(
            "dbg_stage", [16, 2 * T * D], bf16, kind="ExternalOutput"
        )
        dbg_p = nc.dram_tensor("dbg_p", [2, 128, 512], bf16, kind="ExternalOutput")

    RG = [list(range(NCORES))]

    with tile.TileContext(nc) as tc:
        with (
            tc.tile_pool(name="constp", bufs=1) as constp,
            tc.tile_pool(name="qpool", bufs=3) as qpool,
            tc.tile_pool(name="wpool", bufs=1) as wpool,
            tc.tile_pool(name="misc", bufs=1) as misc,
            tc.tile_pool(name="outp", bufs=3) as outp,
            tc.tile_pool(name="pacc", bufs=2, space="PSUM") as pacc,
            tc.tile_pool(name="ptp", bufs=2, space="PSUM") as ptp,
            tc.tile_pool(name="po", bufs=4, space="PSUM") as po,
            tc.tile_pool(name="dram", bufs=1, space="DRAM") as dram,
        ):
            # W: two big SWDGE cast-DMAs (f32 HBM -> bf16 SBUF), fully
            # resident.  Issued first so they stream in the background.
            w_sb = []
            for g2, wsrc in enumerate((wr, wi)):
                wt = wpool.tile([128, T * 4 * 512], bf16, name=f"w{g2}")
                nc.gpsimd.dma_start(
                    out=wt[:].rearrange("p (t n e) -> p t n e", t=T, n=4),
                    in_=wsrc[:],
                )
                w_sb.append(wt)

            # constants (sync ring: fmat/ident now, cmat on scalar ring)
            fmat_sb = constp.tile([128, 32 * 128], f32r)
            nc.sync.dma_start(
                out=fmat_sb[:].rearrange("p (n m) -> p n m", n=32), in_=fmat_d[:]
            )
            ident_sb = constp.tile([128, 128], f32r)
            nc.sync.dma_start(out=ident_sb[:], in_=ident_d[:])
            cmat_sb = constp.tile([128, 2 * L], bf16)
            nc.scalar.dma_start(
                out=cmat_sb[:].rearrange("p (k m) -> p k m", k=2),
                in_=cmat_d[:].rearrange("k p m -> p k m"),
            )

            # ---- step 1 (f32r): X2 = F2 @ qb -> [128 (s,a,t), 512 d]
            x2ps = pacc.tile([128, 512], f32, tag="acc")
            for lo in range(8):  # 1 MB q transfers, 4 l-chunks each
                qt = qpool.tile([128, 4 * 512], f32r, name="qt", tag="qt")
                nc.sync.dma_start(
                    out=qt[:].rearrange("p (n d) -> p n d", n=4),
                    in_=qb[:].rearrange("(n p) d -> p n d", p=128)[
                        :, lo * 4 : (lo + 1) * 4
                    ],
                )
                for li in range(4):
                    gl = lo * 4 + li
                    nc.tensor.matmul(
                        x2ps[:],
                        lhsT=fmat_sb[:, gl * 128 : (gl + 1) * 128],
                        rhs=qt[:, li * 512 : (li + 1) * 512],
                        start=(gl == 0),
                        stop=(gl == 31),
                    )
            x2sb = misc.tile([128, 512], f32r)
            nc.scalar.copy(x2sb[:], x2ps[:])
            if debug:
                nc.sync.dma_start(out=dbg_x2[:], in_=x2sb[:])

            # ---- exchange 1: shard s = my batch's X rows for core s's modes
            b1in = dram.tile([NCORES, 2, T, D], f32r)
            nc.sync.dma_start(
                out=b1in[:].rearrange("s a t d -> (s a t) d"), in_=x2sb[:]
            )
            b1out = dram.tile([NCORES, 2, T, D], f32r)
            nc.gpsimd.collective_compute(
                "AllToAll",
                mybir.AluOpType.bypass,
                replica_groups=RG,
                ins=[b1in.opt()],
                outs=[b1out.opt()],
            )

            # ---- load Xm [128 (j,a,t), 512 d], transpose, cast to bf16
            xm_sb = misc.tile([128, 512], f32r)
            nc.sync.dma_start(
                out=xm_sb[:], in_=b1out[:].rearrange("j a t d -> (j a t) d")
            )
            if debug:
                nc.sync.dma_start(out=dbg_xm[:], in_=xm_sb[:])
            xt_sb = misc.tile([128, 512], bf16)
            for dc in range(4):
                tp = ptp.tile([128, 128], f32r, name="tp", tag="tp")
                nc.tensor.transpose(
                    tp[:], xm_sb[:, dc * 128 : (dc + 1) * 128], ident_sb[:]
                )
                cp = nc.scalar.copy if dc % 2 else nc.vector.tensor_copy
                cp(xt_sb[:, dc * 128 : (dc + 1) * 128], tp[:])

            # ---- step 2 (bf16): per (g2, t): products, M=16 cols m = 2j+a
            xt_v = xt_sb[:].rearrange("p (dc m t) -> p dc m t", dc=4, m=16, t=T)
            stage = misc.tile([16, 2 * T * D], bf16)  # free = (g2, t, e)
            for g2 in range(2):
                for t in range(T):
                    o = po.tile([16, 512], f32, name="o", tag="o")
                    for dc in range(4):
                        nc.tensor.matmul(
                            o[:],
                            lhsT=xt_v[:, dc, :, t],
                            rhs=w_sb[g2][
                                :, (t * 4 + dc) * 512 : (t * 4 + dc + 1) * 512
                            ],
                            start=(dc == 0),
                            stop=(dc == 3),
                        )
                    cp = nc.scalar.copy if (g2 * T + t) % 2 else nc.vector.tensor_copy
                    cp(stage[:, (g2 * T + t) * 512 : (g2 * T + t + 1) * 512], o[:])
            if debug:
                nc.sync.dma_start(out=dbg_stage[:], in_=stage[:])

            # ---- exchange 2: shard j = stage partitions 2j..2j+1, each
            # partition's (g2, t, e) contiguous
            b2in = dram.tile([NCORES, 2, 2, T, D], bf16)
            nc.sync.dma_start(
                out=b2in[:].rearrange("j a g2 t e -> (j a) (g2 t e)"), in_=stage[:]
            )
            b2out = dram.tile([NCORES, 2, 2, T, D], bf16)
            nc.gpsimd.collective_compute(
                "AllToAll",
                mybir.AluOpType.bypass,
                replica_groups=RG,
                ins=[b2in.opt()],
                outs=[b2out.opt()],
            )

            # ---- step 3 (bf16): out = CS4 @ P, K = 2 chunks (rows (j,a,g2,t))
            ps_rhs = []
            for kc in range(2):
                pg = pool_tile = misc.tile([128, 512], bf16, name=f"pg{kc}")
                nc.sync.dma_start(
                    out=pg[:],
                    in_=b2out[kc * 4 : (kc + 1) * 4].rearrange(
                        "j a g t e -> (j a g t) e"
                    ),
                )
                ps_rhs.append(pg)
            if debug:
                nc.sync.dma_start(out=dbg_p[0], in_=ps_rhs[0][:])
                nc.sync.dma_start(out=dbg_p[1], in_=ps_rhs[1][:])
            for m in range(32):
                ps = pacc.tile([128, 512], f32, name="ps3", tag="acc")
                for kc in range(2):
                    nc.tensor.matmul(
                        ps[:],
                        lhsT=cmat_sb[:, kc * L + m * 128 : kc * L + (m + 1) * 128],
                        rhs=ps_rhs[kc][:],
                        start=(kc == 0),
                        stop=(kc == 1),
                    )
                ot = outp.tile([128, 512], f32, name="ot", tag="ot")
                cp = nc.scalar.copy if m % 2 else nc.vector.tensor_copy
                cp(ot[:], ps[:])
                nc.sync.dma_start(out=out[m * 128 : (m + 1) * 128, :], in_=ot[:])

    nc.compile()
    return nc


_NC_CACHE = None


def _get_nc():
    global _NC_CACHE
    if _NC_CACHE is None:
        _NC_CACHE = build_nc()
    return _NC_CACHE


def _prep_w(w, sl):
    # [D, D, M] -> modes sl -> [128, T, 4, 512]: out[p, t, dc, e] = w[dc*128+p, e, t]
    wt = w[:, :, sl]  # [d, e, T]
    wt = wt.reshape(4, 128, 512, T).transpose(1, 3, 0, 2)
    return np.ascontiguousarray(wt, dtype=np.float32)


def run(q, w_real, w_imag, trace=False, debug=False):
    from concourse.bass_utils import run_bass_kernel_spmd

    nc = build_nc(debug=True) if debug else _get_nc()
    q = np.ascontiguousarray(np.asarray(q), dtype=np.float32)
    w_real = np.asarray(w_real)
    w_imag = np.asarray(w_imag)
    fmat_np, cmat_np = _constants()
    ident_np = np.eye(128, dtype=np.float32)
    in_maps = []
    for c in range(NCORES):
        sl = slice(c * T, (c + 1) * T)
        in_maps.append(
            {
                "qb": np.ascontiguousarray(q[c]),
                "wr": _prep_w(w_real, sl),
                "wi": _prep_w(w_imag, sl),
                "fmat": fmat_np,
                "cmat": cmat_np,
                "ident": ident_np,
            }
        )
    res = run_bass_kernel_spmd(
        nc, in_maps, core_ids=list(range(NCORES)), trace=trace
    )
    out = np.stack([r["out"] for r in res.results], axis=0)
    return out, res


def kernel(q, w_real, w_imag):
    out, _ = run(q, w_real, w_imag)
    return out
